# revision 41
# baseline (speedup 1.0000x reference)
"""Causal self-attention (B=4, T=2048, C=1024, H=16, Dh=64) on 8 TRN2 NeuronCores.

Sharding: core c owns batch c//2 and head-group c%2 (8 heads = 512 d-rows of
the output projection contraction).  Host sums the two partials per batch and
adds b_proj.  446.8us (previous session's kernel) -> ~287us.

Per-core structure (all matmuls fp16 with fp32 PSUM accumulation):
  - Q^T stacked per head-pair [128(2x64d), T]; K^T stored PADDED per parity
    (Kev rows 0:64 = even head / 64:128 = zeros, Kod reversed).  Every ST is
    then a full 128-contraction (128,128)-config matmul -- the zero K rows
    annihilate the other head's Q rows -- so QKV/ST/AV/proj all share one PE
    tile config and sustain the 1 col/cycle moving rate (median matmul
    cadence 216ns for 512-wide moving at 2.4GHz).
  - V in [k, d] layout via x-stationary matmuls with 512-wide moving (all 8
    heads per pass), one strided copy per 128-row tile; a ones column at
    d=64 rides each AV matmul to produce the softmax denominator in row 64.
  - Causal masking: diagonal key tiles restrict the ST/AV/EXP/mask moving
    range to queries >= 128*r (fully-masked columns are never computed).
  - Softmax division off the PE critical path: two fast copies free the PSUM
    accumulators, then reciprocal_approx_fast + gpsimd partition_broadcast
    (base partition 0 only!) + one in-place [128,512] multiply.  The final
    boundary multiplies straight out of PSUM since the projection tail
    waits on it.
  - Schedule: QKV chunk qc+1 (and for the last chunk, the three earlier
    projections) are interleaved group-by-group into attention(qc)'s
    per-key-tile loop so the PE never starves while ACT paces the exps.
Output: fp16 partial [T, C] per core (f16 rounding of partials is ~1e-3
absolute, well within the 2e-2 gate).
"""

import sys

if "/opt/trn_rl_repo" not in sys.path:
    sys.path.insert(0, "/opt/trn_rl_repo")

import numpy as np

B, T, C, H, Dh = 4, 2048, 1024, 16, 64
NCORES = 8
HPC = 8                    # heads per core
NP = HPC // 2              # head pairs per core = 4
KT_C = C // 128            # 8 contraction tiles for the projections
TKT = T // 128             # 16 key tiles
QC = T // 512              # 4 query chunks of 512
SCALE = 1.0 / np.sqrt(Dh)
DIAG_RESTRICT = True
DEBUG = False

_cache = {}


def _build(has_bias: bool):
    import concourse.tile as tile
    from concourse import bacc, mybir

    f32 = mybir.dt.float32
    f16 = mybir.dt.float16
    EXP = mybir.ActivationFunctionType.Exp

    nc = bacc.Bacc("TRN2", target_bir_lowering=False, debug=False,
                   num_devices=NCORES)

    xT_d = nc.dram_tensor("xT", [C, T], f16, kind="ExternalInput")
    wqk_d = nc.dram_tensor("w_qk", [C, 2 * HPC * Dh], f16, kind="ExternalInput")
    wv_d = nc.dram_tensor("w_v", [C, HPC * Dh], f16, kind="ExternalInput")
    wp_d = nc.dram_tensor("w_p", [HPC * Dh, C], f16, kind="ExternalInput")
    mask_d = nc.dram_tensor("masks", [128, 4, 512], f16, kind="ExternalInput")
    bqk_d = nc.dram_tensor("b_qk", [128, 2 * NP], f32, kind="ExternalInput")
    bv_d = nc.dram_tensor("b_v_row", [128, HPC * Dh], f32, kind="ExternalInput")
    out_d = nc.dram_tensor("out_p", [T, C], f16, kind="ExternalOutput")
    if DEBUG:
        dbg_yt_d = nc.dram_tensor("dbg_yt", [128, NP, T], f16,
                                  kind="ExternalOutput")
        dbg_q_d = nc.dram_tensor("dbg_q", [128, NP, T], f16,
                                 kind="ExternalOutput")
        dbg_k_d = nc.dram_tensor("dbg_k", [128, NP, T], f16,
                                 kind="ExternalOutput")
        dbg_v_d = nc.dram_tensor("dbg_v", [128, TKT, HPC, Dh + 1], f16,
                                 kind="ExternalOutput")

    xT_t = xT_d.ap().rearrange("(kt p) m -> p kt m", p=128)    # [128, 8, 2048]
    wqk_t = wqk_d.ap().rearrange("(kt p) n -> p kt n", p=128)  # [128, 8, 1024]
    wv_t = wv_d.ap().rearrange("(kt p) n -> p kt n", p=128)    # [128, 8, 512]
    wp_t = wp_d.ap().rearrange("(dg p) n -> p dg n", p=128)    # [128, 4, 1024]

    with tile.TileContext(nc) as tc:
        with tc.tile_pool(name="consts", bufs=1) as consts, \
             tc.tile_pool(name="work", bufs=2) as work, \
             tc.tile_pool(name="psum", bufs=2, space="PSUM") as psum:
            pbuf = obuf = work
            psst = psyt = psum

            # ---- constants / persistent tiles ----
            # DMA order matters: K weights + first x chunk unblock the first
            # matmul group as early as possible.
            NQ = HPC * Dh                                  # 512
            wqk_sb = consts.tile([128, KT_C, 2 * HPC * Dh], f16)
            nc.sync.dma_start(wqk_sb[:, :, NQ:NQ + 128],
                              wqk_t[:, :, NQ:NQ + 128])
            xt_sb = consts.tile([128, KT_C, T], f16)
            for kt in range(KT_C):
                nc.sync.dma_start(xt_sb[:, kt, 0:512], xT_t[:, kt, 0:512])
            for pair in range(1, NP):
                c0 = NQ + pair * 128
                nc.sync.dma_start(wqk_sb[:, :, c0:c0 + 128],
                                  wqk_t[:, :, c0:c0 + 128])
            wv_sb = consts.tile([128, KT_C, HPC * Dh], f16)
            nc.sync.dma_start(wv_sb[:], wv_t)
            nc.sync.dma_start(wqk_sb[:, :, 0:NQ], wqk_t[:, :, 0:NQ])
            mask_sb = consts.tile([128, 4, 512], f16)
            nc.sync.dma_start(mask_sb[:], mask_d.ap())
            for mc in range(1, 4):
                sl = slice(mc * 512, (mc + 1) * 512)
                nc.sync.dma_start(xt_sb[:, :, sl], xT_t[:, :, sl])
            wp_sb = consts.tile([128, NP, C], f16)
            nc.sync.dma_start(wp_sb[:], wp_t)
            if has_bias:
                bqk_sb = consts.tile([128, 2 * NP], f32)
                nc.sync.dma_start(bqk_sb[:], bqk_d.ap())
                bv_sb = consts.tile([128, HPC * Dh], f32)
                nc.sync.dma_start(bv_sb[:], bv_d.ap())

            Qst = consts.tile([128, NP, T], f16)        # rows 0:64 even head
            Kev = consts.tile([128, NP, T], f16)        # rows 64:128 zero
            Kod = consts.tile([128, NP, T], f16)        # rows 0:64 zero
            Vt = consts.tile([128, TKT, HPC, Dh + 1], f16)
            YT = consts.tile([128, NP, T], f16)
            # zero-fill on the scalar engine: it is idle during the initial
            # QKV phase while the DVE drains the projection copies
            nc.scalar.memzero(Kev[64:128, :, :])
            nc.scalar.memzero(Kod[0:64, :, :])
            nc.vector.memset(Vt[:, :, :, Dh:Dh + 1], 1.0)

            # ---------- QKV projection for one 512-row chunk ----------
            def qkv_groups(mc):
                msl = slice(mc * 512, (mc + 1) * 512)

                def k_group(pair):
                    ncol = 512 + pair * 128
                    ps = psum.tile([128, 512], f32, tag="ps")
                    for kt in range(KT_C):
                        nc.tensor.matmul(
                            ps[:], wqk_sb[:, kt, ncol:ncol + 128],
                            xt_sb[:, kt, msl],
                            start=(kt == 0), stop=(kt == KT_C - 1))
                    if has_bias:
                        nc.vector.tensor_scalar_add(
                            Kev[0:64, pair, msl], ps[0:64],
                            bqk_sb[0:64, NP + pair:NP + pair + 1])
                        nc.vector.tensor_scalar_add(
                            Kod[64:128, pair, msl], ps[64:128],
                            bqk_sb[64:128, NP + pair:NP + pair + 1])
                    else:
                        nc.vector.tensor_copy(Kev[0:64, pair, msl], ps[0:64])
                        nc.vector.tensor_copy(Kod[64:128, pair, msl],
                                              ps[64:128])

                def q_group(pair):
                    ncol = pair * 128
                    ps = psum.tile([128, 512], f32, tag="ps")
                    for kt in range(KT_C):
                        nc.tensor.matmul(
                            ps[:], wqk_sb[:, kt, ncol:ncol + 128],
                            xt_sb[:, kt, msl],
                            start=(kt == 0), stop=(kt == KT_C - 1))
                    if has_bias:
                        nc.vector.tensor_scalar_add(
                            Qst[:, pair, msl], ps[:],
                            bqk_sb[:, pair:pair + 1])
                    else:
                        nc.vector.tensor_copy(Qst[:, pair, msl], ps[:])

                def v_group(sub):
                    m0 = mc * 512 + sub * 128
                    ktile = 4 * mc + sub
                    vps = psum.tile([128, 512], f32, tag="ps")
                    for kt in range(KT_C):
                        nc.tensor.matmul(
                            vps[:], xt_sb[:, kt, m0:m0 + 128],
                            wv_sb[:, kt, :],
                            start=(kt == 0), stop=(kt == KT_C - 1))
                    vdst = Vt[:, ktile, :, 0:Dh]        # [128, 8, 64] strided
                    vsrc = vps[:].rearrange("p (h d) -> p h d", h=HPC)
                    if has_bias:
                        nc.vector.tensor_add(
                            vdst, vsrc,
                            bv_sb[:].rearrange("p (h d) -> p h d", h=HPC))
                    else:
                        nc.vector.tensor_copy(vdst, vsrc)

                for pair in range(NP):
                    yield lambda p=pair: k_group(p)
                for sub in range(4):
                    yield lambda s=sub: v_group(s)
                for pair in range(NP):
                    yield lambda p=pair: q_group(p)

            # ---------- projection of one 512-row chunk (after attn qc) ----
            def proj_groups(qc):
                def pgroup(mt, nh):
                    pp2 = psum.tile([128, 512], f32, tag="ps")
                    for dg in range(NP):
                        nc.tensor.matmul(
                            pp2[:], YT[:, dg, mt * 128:(mt + 1) * 128],
                            wp_sb[:, dg, nh * 512:(nh + 1) * 512],
                            start=(dg == 0), stop=(dg == NP - 1))
                    ot = obuf.tile([128, 512], f16, tag="ot", bufs=4)
                    if (mt + nh) % 2:
                        nc.scalar.copy(ot[:], pp2[:])
                    else:
                        nc.vector.tensor_copy(ot[:], pp2[:])
                    nc.sync.dma_start(
                        out_d.ap()[mt * 128:(mt + 1) * 128,
                                   nh * 512:(nh + 1) * 512], ot[:])

                for mt in range(4 * qc, 4 * qc + 4):
                    for nh in range(2):
                        yield lambda m=mt, n=nh: pgroup(m, n)

            # ---------- attention for one query chunk, with bg interleave --
            def attention(qc, background):
                q_sl = slice(qc * 512, (qc + 1) * 512)
                nkt = 4 * (qc + 1)
                for hp in range(NP):
                    yt0 = psyt.tile([65, 512], f32, tag="yt")
                    yt1 = psyt.tile([65, 512], f32, tag="yt")
                    for kt in range(nkt):
                        k_sl = slice(kt * 128, (kt + 1) * 128)
                        r = kt - 4 * qc
                        # queries before 128*r in this chunk see no valid key
                        # in a diagonal tile: skip those moving columns.
                        q0 = max(r, 0) * 128 if DIAG_RESTRICT else 0
                        qr = slice(qc * 512 + q0, (qc + 1) * 512)
                        cr = slice(q0, 512)
                        stp = psst.tile([128, 2, 512], f32, tag="st")
                        nc.tensor.matmul(stp[:, 0, cr], Kev[:, hp, k_sl],
                                         Qst[:, hp, qr],
                                         start=True, stop=True)
                        nc.tensor.matmul(stp[:, 1, cr], Kod[:, hp, k_sl],
                                         Qst[:, hp, qr],
                                         start=True, stop=True)
                        pp = pbuf.tile([128, 2, 512], f16, tag="pp", bufs=8)
                        nc.scalar.activation(pp[:, :, cr], stp[:, :, cr],
                                             EXP, scale=SCALE)
                        if r >= 0:                       # diagonal: mask
                            nc.vector.tensor_mul(
                                pp[:, :, cr], pp[:, :, cr],
                                mask_sb[:, r:r + 1, cr].broadcast_to(
                                    [128, 2, 512 - q0]))
                        first, last = (kt == 0), (kt == nkt - 1)
                        nc.tensor.matmul(yt0[:, cr], Vt[:, kt, 2 * hp, :],
                                         pp[:, 0, cr], start=first, stop=last,
                                         skip_group_check=True)
                        nc.tensor.matmul(yt1[:, cr], Vt[:, kt, 2 * hp + 1, :],
                                         pp[:, 1, cr], start=first, stop=last,
                                         skip_group_check=True)
                        if background:
                            background.pop(0)()
                    # Free the yt accumulators with copies, then normalize
                    # YT in place.  partition_broadcast only writes correctly
                    # at base partition 0, so broadcast both heads there and
                    # shift-copy the odd half up with the DVE.  The very last
                    # boundary instead multiplies straight out of PSUM (the
                    # projection tail waits on it).
                    last = (qc == QC - 1 and hp == NP - 1)
                    if not last:
                        bc = work.tile([128, 512], f32, tag="bc", bufs=2)
                        bcx = work.tile([64, 512], f32, tag="bcx", bufs=2)
                        for h, ytp in ((0, yt0), (1, yt1)):
                            y_sl = (slice(h * 64, (h + 1) * 64), hp, q_sl)
                            zr = work.tile([1, 512], f32, tag="zr", bufs=4)
                            nc.vector.tensor_copy(zr[:], ytp[64:65, :])
                            nc.vector.tensor_copy(YT[y_sl], ytp[0:64, :])
                            zi = work.tile([1, 512], f32, tag="zi", bufs=4)
                            nc.vector.reciprocal_approx_fast(zi[:], zr[:])
                            nc.gpsimd.partition_broadcast(
                                bc[0:64, :] if h == 0 else bcx[:], zi[:])
                        nc.vector.tensor_copy(bc[64:128, :], bcx[:])
                        nc.vector.tensor_mul(YT[:, hp, q_sl],
                                             YT[:, hp, q_sl], bc[:])
                    else:
                        # final boundary gates the projection tail: multiply
                        # straight out of PSUM in column halves so proj(mt)
                        # for the first half can start earlier
                        for ci in range(2):
                            cs = slice(ci * 256, (ci + 1) * 256)
                            qs = slice(qc * 512 + ci * 256,
                                       qc * 512 + (ci + 1) * 256)
                            for h, ytp in ((0, yt0), (1, yt1)):
                                y_sl = (slice(h * 64, (h + 1) * 64), hp, qs)
                                zr = work.tile([1, 256], f32, tag="zr",
                                               bufs=4)
                                nc.vector.tensor_copy(zr[:], ytp[64:65, cs])
                                zi = work.tile([1, 256], f32, tag="zi",
                                               bufs=4)
                                nc.vector.reciprocal_approx_fast(zi[:], zr[:])
                                bch = work.tile([64, 256], f32, tag="bch",
                                                bufs=4)
                                nc.gpsimd.partition_broadcast(bch[:], zi[:])
                                nc.vector.tensor_mul(YT[y_sl], ytp[0:64, cs],
                                                     bch[:])

            # ---------------- main schedule ----------------
            # attn(qc) is paced by ACT exp; fill the PE with qkv chunk qc+1,
            # and all three early projections during the long attn(3).
            for g in qkv_groups(0):
                g()
            for qc in range(QC):
                background = []
                if qc + 1 < QC:
                    background.extend(qkv_groups(qc + 1))
                else:
                    for p in range(QC - 1):
                        background.extend(proj_groups(p))
                attention(qc, background)
                for g in background:                     # leftovers
                    g()
            for g in proj_groups(QC - 1):
                g()
            if DEBUG:
                nc.sync.dma_start(dbg_yt_d.ap(), YT[:])
                nc.sync.dma_start(dbg_q_d.ap(), Qst[:])
                nc.sync.dma_start(dbg_k_d.ap(), Kev[:])
                nc.sync.dma_start(dbg_v_d.ap(), Vt[:])

    nc.compile()
    return nc


def _get_nc(has_bias: bool):
    key = ("nc", has_bias)
    if key not in _cache:
        _cache[key] = _build(has_bias)
    return _cache[key]


def _make_masks() -> np.ndarray:
    # masks[p, r, h, q] = 1.0 where key (128*r + p) <= query q in a 512-chunk
    p = np.arange(128)[:, None, None]
    r = np.arange(4)[None, :, None]
    q = np.arange(512)[None, None, :]
    m = ((128 * r + p) <= q).astype(np.float16)           # [128, 4, 512]
    return np.ascontiguousarray(m)


def kernel(x, W_qkv, b_qkv, W_proj, b_proj):
    from concourse.bass_utils import run_bass_kernel_spmd

    x = np.asarray(x, dtype=np.float32)
    W_qkv = np.asarray(W_qkv, dtype=np.float32)
    b_qkv = np.asarray(b_qkv, dtype=np.float32)
    W_proj = np.asarray(W_proj, dtype=np.float32)
    b_proj = np.asarray(b_proj, dtype=np.float32)

    has_bias = bool(np.any(b_qkv != 0.0))
    nc = _get_nc(has_bias)

    masks = _make_masks()
    GW = HPC * Dh                                         # 512 channels/group
    in_maps = []
    for c in range(NCORES):
        b, g = divmod(c, 2)
        h0 = g * GW
        xT = np.ascontiguousarray(x[b].T.astype(np.float16))   # [C, T]
        w_q = W_qkv[:, h0:h0 + GW]
        w_k = W_qkv[:, C + h0:C + h0 + GW]
        w_v = W_qkv[:, 2 * C + h0:2 * C + h0 + GW]
        b_q = b_qkv[h0:h0 + GW]
        b_k = b_qkv[C + h0:C + h0 + GW]
        b_v = b_qkv[2 * C + h0:2 * C + h0 + GW]
        in_maps.append({
            "xT": xT,
            "w_qk": np.ascontiguousarray(
                np.concatenate([w_q, w_k], axis=1).astype(np.float16)),
            "w_v": np.ascontiguousarray(w_v.astype(np.float16)),
            "w_p": np.ascontiguousarray(
                W_proj[h0:h0 + GW, :].astype(np.float16)),
            "masks": masks,
            "b_qk": np.ascontiguousarray(
                np.concatenate([b_q.reshape(NP, 128).T,
                                b_k.reshape(NP, 128).T], axis=1)),
            "b_v_row": np.ascontiguousarray(
                np.broadcast_to(b_v[None, :], (128, GW)).astype(np.float32)),
        })

    res = run_bass_kernel_spmd(nc, in_maps, core_ids=list(range(NCORES)),
                               **_cache.get("run_kwargs", {}))
    _cache["last_results"] = res

    out = np.empty((B, T, C), dtype=np.float32)
    for b in range(B):
        out[b] = (res.results[2 * b]["out_p"].astype(np.float32)
                  + res.results[2 * b + 1]["out_p"].astype(np.float32)
                  + b_proj[None, :])
    return out



# revision 42
# speedup vs baseline: 1.1926x; 1.1926x over previous
"""Causal self-attention (B=4, T=2048, C=1024, H=16, Dh=64) on 8 TRN2 NeuronCores.

Sharding: core c owns batch c//2 and head-group c%2 (8 heads = 512 d-rows of
the output projection contraction).  Host sums the two partials per batch and
adds b_proj.  446.8us (previous session's kernel) -> ~287us.

Per-core structure (all matmuls fp16 with fp32 PSUM accumulation):
  - Q^T stacked per head-pair [128(2x64d), T]; K^T stored PADDED per parity
    (Kev rows 0:64 = even head / 64:128 = zeros, Kod reversed).  Every ST is
    then a full 128-contraction (128,128)-config matmul -- the zero K rows
    annihilate the other head's Q rows -- so QKV/ST/AV/proj all share one PE
    tile config and sustain the 1 col/cycle moving rate (median matmul
    cadence 216ns for 512-wide moving at 2.4GHz).
  - V in [k, d] layout via x-stationary matmuls with 512-wide moving (all 8
    heads per pass), one strided copy per 128-row tile; a ones column at
    d=64 rides each AV matmul to produce the softmax denominator in row 64.
  - Causal masking: diagonal key tiles restrict the ST/AV/EXP/mask moving
    range to queries >= 128*r (fully-masked columns are never computed).
  - Softmax division off the PE critical path: two fast copies free the PSUM
    accumulators, then reciprocal_approx_fast + gpsimd partition_broadcast
    (base partition 0 only!) + one in-place [128,512] multiply.  The final
    boundary multiplies straight out of PSUM since the projection tail
    waits on it.
  - Schedule: QKV chunk qc+1 (and for the last chunk, the three earlier
    projections) are interleaved group-by-group into attention(qc)'s
    per-key-tile loop so the PE never starves while ACT paces the exps.
Output: fp16 partial [T, C] per core (f16 rounding of partials is ~1e-3
absolute, well within the 2e-2 gate).
"""

import sys

if "/opt/trn_rl_repo" not in sys.path:
    sys.path.insert(0, "/opt/trn_rl_repo")

import numpy as np

B, T, C, H, Dh = 4, 2048, 1024, 16, 64
NCORES = 8
HPC = 8                    # heads per core
NP = HPC // 2              # head pairs per core = 4
KT_C = C // 128            # 8 contraction tiles for the projections
TKT = T // 128             # 16 key tiles
QC = T // 512              # 4 query chunks of 512
SCALE = 1.0 / np.sqrt(Dh)
DIAG_RESTRICT = True
DEBUG = False

_cache = {}


def _build(has_bias: bool):
    import concourse.tile as tile
    from concourse import bacc, mybir

    f32 = mybir.dt.float32
    f16 = mybir.dt.float16
    EXP = mybir.ActivationFunctionType.Exp

    nc = bacc.Bacc("TRN2", target_bir_lowering=False, debug=False,
                   num_devices=NCORES)

    xT_d = nc.dram_tensor("xT", [C, T], f16, kind="ExternalInput")
    wqk_d = nc.dram_tensor("w_qk", [C, 2 * HPC * Dh], f16, kind="ExternalInput")
    wv_d = nc.dram_tensor("w_v", [C, HPC * Dh], f16, kind="ExternalInput")
    wp_d = nc.dram_tensor("w_p", [HPC * Dh, C], f16, kind="ExternalInput")
    mask_d = nc.dram_tensor("masks", [128, 4, 2, 512], f16, kind="ExternalInput")
    bqk_d = nc.dram_tensor("b_qk", [128, 2 * NP], f32, kind="ExternalInput")
    bv_d = nc.dram_tensor("b_v_row", [128, HPC * Dh], f32, kind="ExternalInput")
    out_d = nc.dram_tensor("out_p", [T, C], f16, kind="ExternalOutput")
    if DEBUG:
        dbg_yt_d = nc.dram_tensor("dbg_yt", [128, NP, T], f16,
                                  kind="ExternalOutput")
        dbg_q_d = nc.dram_tensor("dbg_q", [128, NP, T], f16,
                                 kind="ExternalOutput")
        dbg_k_d = nc.dram_tensor("dbg_k", [128, NP, T], f16,
                                 kind="ExternalOutput")
        dbg_v_d = nc.dram_tensor("dbg_v", [128, TKT, HPC, Dh + 1], f16,
                                 kind="ExternalOutput")

    xT_t = xT_d.ap().rearrange("(kt p) m -> p kt m", p=128)    # [128, 8, 2048]
    wqk_t = wqk_d.ap().rearrange("(kt p) n -> p kt n", p=128)  # [128, 8, 1024]
    wv_t = wv_d.ap().rearrange("(kt p) n -> p kt n", p=128)    # [128, 8, 512]
    wp_t = wp_d.ap().rearrange("(dg p) n -> p dg n", p=128)    # [128, 4, 1024]

    with tile.TileContext(nc) as tc:
        with tc.tile_pool(name="consts", bufs=1) as consts, \
             tc.tile_pool(name="work", bufs=2) as work, \
             tc.tile_pool(name="psum", bufs=2, space="PSUM") as psum:
            pbuf = obuf = work
            psst = psyt = psum

            # ---- constants / persistent tiles ----
            # DMA order matters: K weights + first x chunk unblock the first
            # matmul group as early as possible.
            NQ = HPC * Dh                                  # 512
            wqk_sb = consts.tile([128, KT_C, 2 * HPC * Dh], f16)
            nc.sync.dma_start(wqk_sb[:, :, NQ:NQ + 128],
                              wqk_t[:, :, NQ:NQ + 128])
            xt_sb = consts.tile([128, KT_C, T], f16)
            for kt in range(KT_C):
                nc.sync.dma_start(xt_sb[:, kt, 0:512], xT_t[:, kt, 0:512])
            for pair in range(1, NP):
                c0 = NQ + pair * 128
                nc.sync.dma_start(wqk_sb[:, :, c0:c0 + 128],
                                  wqk_t[:, :, c0:c0 + 128])
            wv_sb = consts.tile([128, KT_C, HPC * Dh], f16)
            nc.sync.dma_start(wv_sb[:], wv_t)
            nc.sync.dma_start(wqk_sb[:, :, 0:NQ], wqk_t[:, :, 0:NQ])
            mask_sb = consts.tile([128, 4, 2, 512], f16)
            nc.sync.dma_start(mask_sb[:], mask_d.ap())
            for mc in range(1, 4):
                sl = slice(mc * 512, (mc + 1) * 512)
                nc.sync.dma_start(xt_sb[:, :, sl], xT_t[:, :, sl])
            wp_sb = consts.tile([128, NP, C], f16)
            nc.sync.dma_start(wp_sb[:], wp_t)
            if has_bias:
                bqk_sb = consts.tile([128, 2 * NP], f32)
                nc.sync.dma_start(bqk_sb[:], bqk_d.ap())
                bv_sb = consts.tile([128, HPC * Dh], f32)
                nc.sync.dma_start(bv_sb[:], bv_d.ap())

            Qst = consts.tile([128, NP, T], f16)        # rows 0:64 even head
            Kev = consts.tile([128, NP, T], f16)        # rows 64:128 zero
            Kod = consts.tile([128, NP, T], f16)        # rows 0:64 zero
            Vt = consts.tile([128, TKT, HPC, Dh + 1], f16)
            YT = consts.tile([128, NP, T], f16)
            # zero-fill on the scalar engine: it is idle during the initial
            # QKV phase while the DVE drains the projection copies
            nc.scalar.memzero(Kev[64:128, :, :])
            nc.scalar.memzero(Kod[0:64, :, :])
            nc.vector.memset(Vt[:, :, :, Dh:Dh + 1], 1.0)

            # ---------- QKV projection for one 512-row chunk ----------
            def qkv_groups(mc):
                msl = slice(mc * 512, (mc + 1) * 512)

                def k_group(pair):
                    ncol = 512 + pair * 128
                    ps = psum.tile([128, 512], f32, tag="ps")
                    for kt in range(KT_C):
                        nc.tensor.matmul(
                            ps[:], wqk_sb[:, kt, ncol:ncol + 128],
                            xt_sb[:, kt, msl],
                            start=(kt == 0), stop=(kt == KT_C - 1))
                    if has_bias:
                        nc.vector.tensor_scalar_add(
                            Kev[0:64, pair, msl], ps[0:64],
                            bqk_sb[0:64, NP + pair:NP + pair + 1])
                        nc.vector.tensor_scalar_add(
                            Kod[64:128, pair, msl], ps[64:128],
                            bqk_sb[64:128, NP + pair:NP + pair + 1])
                    else:
                        nc.vector.tensor_copy(Kev[0:64, pair, msl], ps[0:64])
                        nc.vector.tensor_copy(Kod[64:128, pair, msl],
                                              ps[64:128])

                def q_group(pair):
                    ncol = pair * 128
                    ps = psum.tile([128, 512], f32, tag="ps")
                    for kt in range(KT_C):
                        nc.tensor.matmul(
                            ps[:], wqk_sb[:, kt, ncol:ncol + 128],
                            xt_sb[:, kt, msl],
                            start=(kt == 0), stop=(kt == KT_C - 1))
                    if has_bias:
                        nc.vector.tensor_scalar_add(
                            Qst[:, pair, msl], ps[:],
                            bqk_sb[:, pair:pair + 1])
                    else:
                        nc.vector.tensor_copy(Qst[:, pair, msl], ps[:])

                def v_group(sub):
                    m0 = mc * 512 + sub * 128
                    ktile = 4 * mc + sub
                    vps = psum.tile([128, 512], f32, tag="ps")
                    for kt in range(KT_C):
                        nc.tensor.matmul(
                            vps[:], xt_sb[:, kt, m0:m0 + 128],
                            wv_sb[:, kt, :],
                            start=(kt == 0), stop=(kt == KT_C - 1))
                    vdst = Vt[:, ktile, :, 0:Dh]        # [128, 8, 64] strided
                    vsrc = vps[:].rearrange("p (h d) -> p h d", h=HPC)
                    if has_bias:
                        nc.vector.tensor_add(
                            vdst, vsrc,
                            bv_sb[:].rearrange("p (h d) -> p h d", h=HPC))
                    else:
                        nc.vector.tensor_copy(vdst, vsrc)

                for pair in range(NP):
                    yield lambda p=pair: k_group(p)
                for sub in range(4):
                    yield lambda s=sub: v_group(s)
                for pair in range(NP):
                    yield lambda p=pair: q_group(p)

            # ---------- projection of one 512-row chunk (after attn qc) ----
            def proj_groups(qc):
                def pgroup(mt, nh):
                    pp2 = psum.tile([128, 512], f32, tag="ps")
                    for dg in range(NP):
                        nc.tensor.matmul(
                            pp2[:], YT[:, dg, mt * 128:(mt + 1) * 128],
                            wp_sb[:, dg, nh * 512:(nh + 1) * 512],
                            start=(dg == 0), stop=(dg == NP - 1))
                    ot = obuf.tile([128, 512], f16, tag="ot", bufs=4)
                    if (mt + nh) % 2:
                        nc.scalar.copy(ot[:], pp2[:])
                    else:
                        nc.vector.tensor_copy(ot[:], pp2[:])
                    nc.sync.dma_start(
                        out_d.ap()[mt * 128:(mt + 1) * 128,
                                   nh * 512:(nh + 1) * 512], ot[:])

                for mt in range(4 * qc, 4 * qc + 4):
                    for nh in range(2):
                        yield lambda m=mt, n=nh: pgroup(m, n)

            # ---------- attention for one query chunk, with bg interleave --
            def attention(qc, background):
                q_sl = slice(qc * 512, (qc + 1) * 512)
                nkt = 4 * (qc + 1)
                for hp in range(NP):
                    yt0 = psyt.tile([65, 512], f32, tag="yt")
                    yt1 = psyt.tile([65, 512], f32, tag="yt")
                    for kt in range(nkt):
                        k_sl = slice(kt * 128, (kt + 1) * 128)
                        r = kt - 4 * qc
                        # queries before 128*r in this chunk see no valid key
                        # in a diagonal tile: skip those moving columns.
                        q0 = max(r, 0) * 128 if DIAG_RESTRICT else 0
                        qr = slice(qc * 512 + q0, (qc + 1) * 512)
                        cr = slice(q0, 512)
                        stp = psst.tile([128, 2, 512], f32, tag="st")
                        nc.tensor.matmul(stp[:, 0, cr], Kev[:, hp, k_sl],
                                         Qst[:, hp, qr],
                                         start=True, stop=True)
                        nc.tensor.matmul(stp[:, 1, cr], Kod[:, hp, k_sl],
                                         Qst[:, hp, qr],
                                         start=True, stop=True)
                        pp = pbuf.tile([128, 2, 512], f16, tag="pp", bufs=8)
                        nc.scalar.activation(pp[:, :, cr], stp[:, :, cr],
                                             EXP, scale=SCALE)
                        if r >= 0:                       # diagonal: mask
                            nc.vector.tensor_mul(pp[:, :, cr], pp[:, :, cr],
                                                 mask_sb[:, r, :, cr])
                        first, last = (kt == 0), (kt == nkt - 1)
                        nc.tensor.matmul(yt0[:, cr], Vt[:, kt, 2 * hp, :],
                                         pp[:, 0, cr], start=first, stop=last,
                                         skip_group_check=True)
                        nc.tensor.matmul(yt1[:, cr], Vt[:, kt, 2 * hp + 1, :],
                                         pp[:, 1, cr], start=first, stop=last,
                                         skip_group_check=True)
                        if background:
                            background.pop(0)()
                    # Free the yt accumulators with copies, then normalize
                    # YT in place.  partition_broadcast only writes correctly
                    # at base partition 0, so broadcast both heads there and
                    # shift-copy the odd half up with the DVE.  The very last
                    # boundary instead multiplies straight out of PSUM (the
                    # projection tail waits on it).
                    last = (qc == QC - 1 and hp == NP - 1)
                    if not last:
                        bc = work.tile([128, 512], f32, tag="bc", bufs=2)
                        bcx = work.tile([64, 512], f32, tag="bcx", bufs=2)
                        for h, ytp in ((0, yt0), (1, yt1)):
                            y_sl = (slice(h * 64, (h + 1) * 64), hp, q_sl)
                            zr = work.tile([1, 512], f32, tag="zr", bufs=4)
                            nc.vector.tensor_copy(zr[:], ytp[64:65, :])
                            nc.vector.tensor_copy(YT[y_sl], ytp[0:64, :])
                            zi = work.tile([1, 512], f32, tag="zi", bufs=4)
                            nc.vector.reciprocal_approx_fast(zi[:], zr[:])
                            nc.gpsimd.partition_broadcast(
                                bc[0:64, :] if h == 0 else bcx[:], zi[:])
                        nc.vector.tensor_copy(bc[64:128, :], bcx[:])
                        nc.vector.tensor_mul(YT[:, hp, q_sl],
                                             YT[:, hp, q_sl], bc[:])
                    else:
                        # final boundary gates the projection tail: multiply
                        # straight out of PSUM in column halves so proj(mt)
                        # for the first half can start earlier
                        for ci in range(2):
                            cs = slice(ci * 256, (ci + 1) * 256)
                            qs = slice(qc * 512 + ci * 256,
                                       qc * 512 + (ci + 1) * 256)
                            for h, ytp in ((0, yt0), (1, yt1)):
                                y_sl = (slice(h * 64, (h + 1) * 64), hp, qs)
                                zr = work.tile([1, 256], f32, tag="zr",
                                               bufs=4)
                                nc.vector.tensor_copy(zr[:], ytp[64:65, cs])
                                zi = work.tile([1, 256], f32, tag="zi",
                                               bufs=4)
                                nc.vector.reciprocal_approx_fast(zi[:], zr[:])
                                bch = work.tile([64, 256], f32, tag="bch",
                                                bufs=4)
                                nc.gpsimd.partition_broadcast(bch[:], zi[:])
                                nc.vector.tensor_mul(YT[y_sl], ytp[0:64, cs],
                                                     bch[:])

            # ---------------- main schedule ----------------
            # attn(qc) is paced by ACT exp; fill the PE with qkv chunk qc+1,
            # and all three early projections during the long attn(3).
            for g in qkv_groups(0):
                g()
            for qc in range(QC):
                background = []
                if qc + 1 < QC:
                    background.extend(qkv_groups(qc + 1))
                else:
                    for p in range(QC - 1):
                        background.extend(proj_groups(p))
                attention(qc, background)
                for g in background:                     # leftovers
                    g()
            for g in proj_groups(QC - 1):
                g()
            if DEBUG:
                nc.sync.dma_start(dbg_yt_d.ap(), YT[:])
                nc.sync.dma_start(dbg_q_d.ap(), Qst[:])
                nc.sync.dma_start(dbg_k_d.ap(), Kev[:])
                nc.sync.dma_start(dbg_v_d.ap(), Vt[:])

    nc.compile()
    return nc


def _get_nc(has_bias: bool):
    key = ("nc", has_bias)
    if key not in _cache:
        _cache[key] = _build(has_bias)
    return _cache[key]


def _make_masks() -> np.ndarray:
    # masks[p, r, h, q] = 1.0 where key (128*r + p) <= query q in a 512-chunk
    p = np.arange(128)[:, None, None]
    r = np.arange(4)[None, :, None]
    q = np.arange(512)[None, None, :]
    m = ((128 * r + p) <= q).astype(np.float16)           # [128, 4, 512]
    return np.ascontiguousarray(np.repeat(m[:, :, None, :], 2, axis=2))


def kernel(x, W_qkv, b_qkv, W_proj, b_proj):
    from concourse.bass_utils import run_bass_kernel_spmd

    x = np.asarray(x, dtype=np.float32)
    W_qkv = np.asarray(W_qkv, dtype=np.float32)
    b_qkv = np.asarray(b_qkv, dtype=np.float32)
    W_proj = np.asarray(W_proj, dtype=np.float32)
    b_proj = np.asarray(b_proj, dtype=np.float32)

    has_bias = bool(np.any(b_qkv != 0.0))
    nc = _get_nc(has_bias)

    masks = _make_masks()
    GW = HPC * Dh                                         # 512 channels/group
    in_maps = []
    for c in range(NCORES):
        b, g = divmod(c, 2)
        h0 = g * GW
        xT = np.ascontiguousarray(x[b].T.astype(np.float16))   # [C, T]
        w_q = W_qkv[:, h0:h0 + GW]
        w_k = W_qkv[:, C + h0:C + h0 + GW]
        w_v = W_qkv[:, 2 * C + h0:2 * C + h0 + GW]
        b_q = b_qkv[h0:h0 + GW]
        b_k = b_qkv[C + h0:C + h0 + GW]
        b_v = b_qkv[2 * C + h0:2 * C + h0 + GW]
        in_maps.append({
            "xT": xT,
            "w_qk": np.ascontiguousarray(
                np.concatenate([w_q, w_k], axis=1).astype(np.float16)),
            "w_v": np.ascontiguousarray(w_v.astype(np.float16)),
            "w_p": np.ascontiguousarray(
                W_proj[h0:h0 + GW, :].astype(np.float16)),
            "masks": masks,
            "b_qk": np.ascontiguousarray(
                np.concatenate([b_q.reshape(NP, 128).T,
                                b_k.reshape(NP, 128).T], axis=1)),
            "b_v_row": np.ascontiguousarray(
                np.broadcast_to(b_v[None, :], (128, GW)).astype(np.float32)),
        })

    res = run_bass_kernel_spmd(nc, in_maps, core_ids=list(range(NCORES)),
                               **_cache.get("run_kwargs", {}))
    _cache["last_results"] = res

    out = np.empty((B, T, C), dtype=np.float32)
    for b in range(B):
        out[b] = (res.results[2 * b]["out_p"].astype(np.float32)
                  + res.results[2 * b + 1]["out_p"].astype(np.float32)
                  + b_proj[None, :])
    return out



# revision 43
# speedup vs baseline: 1.2166x; 1.0201x over previous
"""Causal self-attention (B=4, T=2048, C=1024, H=16, Dh=64) on 8 TRN2 NeuronCores.

Sharding: core c owns batch c//2 and head-group c%2 (8 heads = 512 d-rows of
the output projection contraction).  Host sums the two partials per batch and
adds b_proj.  446.8us (previous session's kernel) -> ~287us.

Per-core structure (all matmuls fp16 with fp32 PSUM accumulation):
  - Q^T stacked per head-pair [128(2x64d), T]; K^T stored PADDED per parity
    (Kev rows 0:64 = even head / 64:128 = zeros, Kod reversed).  Every ST is
    then a full 128-contraction (128,128)-config matmul -- the zero K rows
    annihilate the other head's Q rows -- so QKV/ST/AV/proj all share one PE
    tile config and sustain the 1 col/cycle moving rate (median matmul
    cadence 216ns for 512-wide moving at 2.4GHz).
  - V in [k, d] layout via x-stationary matmuls with 512-wide moving (all 8
    heads per pass), one strided copy per 128-row tile; a ones column at
    d=64 rides each AV matmul to produce the softmax denominator in row 64.
  - Causal masking: diagonal key tiles restrict the ST/AV/EXP/mask moving
    range to queries >= 128*r (fully-masked columns are never computed).
  - Softmax division off the PE critical path: two fast copies free the PSUM
    accumulators, then reciprocal_approx_fast + gpsimd partition_broadcast
    (base partition 0 only!) + one in-place [128,512] multiply.  The final
    boundary multiplies straight out of PSUM since the projection tail
    waits on it.
  - Schedule: QKV chunk qc+1 (and for the last chunk, the three earlier
    projections) are interleaved group-by-group into attention(qc)'s
    per-key-tile loop so the PE never starves while ACT paces the exps.
Output: fp16 partial [T, C] per core (f16 rounding of partials is ~1e-3
absolute, well within the 2e-2 gate).
"""

import sys

if "/opt/trn_rl_repo" not in sys.path:
    sys.path.insert(0, "/opt/trn_rl_repo")

import numpy as np

B, T, C, H, Dh = 4, 2048, 1024, 16, 64
NCORES = 8
HPC = 8                    # heads per core
NP = HPC // 2              # head pairs per core = 4
KT_C = C // 128            # 8 contraction tiles for the projections
TKT = T // 128             # 16 key tiles
QC = T // 512              # 4 query chunks of 512
SCALE = 1.0 / np.sqrt(Dh)
DIAG_RESTRICT = True
DEBUG = False

_cache = {}


def _build(has_bias: bool):
    import concourse.tile as tile
    from concourse import bacc, mybir

    f32 = mybir.dt.float32
    f16 = mybir.dt.float16
    EXP = mybir.ActivationFunctionType.Exp

    nc = bacc.Bacc("TRN2", target_bir_lowering=False, debug=False,
                   num_devices=NCORES)

    xT_d = nc.dram_tensor("xT", [C, T], f16, kind="ExternalInput")
    wqk_d = nc.dram_tensor("w_qk", [C, 2 * HPC * Dh], f16, kind="ExternalInput")
    wv_d = nc.dram_tensor("w_v", [C, HPC * Dh], f16, kind="ExternalInput")
    wp_d = nc.dram_tensor("w_p", [HPC * Dh, C], f16, kind="ExternalInput")
    mask_d = nc.dram_tensor("masks", [128, 2, 128], f16, kind="ExternalInput")
    bqk_d = nc.dram_tensor("b_qk", [128, 2 * NP], f32, kind="ExternalInput")
    bv_d = nc.dram_tensor("b_v_row", [128, HPC * Dh], f32, kind="ExternalInput")
    out_d = nc.dram_tensor("out_p", [T, C], f16, kind="ExternalOutput")
    if DEBUG:
        dbg_yt_d = nc.dram_tensor("dbg_yt", [128, NP, T], f16,
                                  kind="ExternalOutput")
        dbg_q_d = nc.dram_tensor("dbg_q", [128, NP, T], f16,
                                 kind="ExternalOutput")
        dbg_k_d = nc.dram_tensor("dbg_k", [128, NP, T], f16,
                                 kind="ExternalOutput")
        dbg_v_d = nc.dram_tensor("dbg_v", [128, TKT, HPC, Dh + 1], f16,
                                 kind="ExternalOutput")

    xT_t = xT_d.ap().rearrange("(kt p) m -> p kt m", p=128)    # [128, 8, 2048]
    wqk_t = wqk_d.ap().rearrange("(kt p) n -> p kt n", p=128)  # [128, 8, 1024]
    wv_t = wv_d.ap().rearrange("(kt p) n -> p kt n", p=128)    # [128, 8, 512]
    wp_t = wp_d.ap().rearrange("(dg p) n -> p dg n", p=128)    # [128, 4, 1024]

    with tile.TileContext(nc) as tc:
        with tc.tile_pool(name="consts", bufs=1) as consts, \
             tc.tile_pool(name="work", bufs=2) as work, \
             tc.tile_pool(name="psum", bufs=2, space="PSUM") as psum:
            pbuf = obuf = work
            psst = psyt = psum

            # ---- constants / persistent tiles ----
            # DMA order matters: K weights + first x chunk unblock the first
            # matmul group as early as possible.
            NQ = HPC * Dh                                  # 512
            wqk_sb = consts.tile([128, KT_C, 2 * HPC * Dh], f16)
            nc.sync.dma_start(wqk_sb[:, :, NQ:NQ + 128],
                              wqk_t[:, :, NQ:NQ + 128])
            xt_sb = consts.tile([128, KT_C, T], f16)
            for kt in range(KT_C):
                nc.sync.dma_start(xt_sb[:, kt, 0:512], xT_t[:, kt, 0:512])
            for pair in range(1, NP):
                c0 = NQ + pair * 128
                nc.sync.dma_start(wqk_sb[:, :, c0:c0 + 128],
                                  wqk_t[:, :, c0:c0 + 128])
            wv_sb = consts.tile([128, KT_C, HPC * Dh], f16)
            nc.sync.dma_start(wv_sb[:], wv_t)
            nc.sync.dma_start(wqk_sb[:, :, 0:NQ], wqk_t[:, :, 0:NQ])
            mask_sb = consts.tile([128, 2, 128], f16)
            nc.sync.dma_start(mask_sb[:], mask_d.ap())
            for mc in range(1, 4):
                sl = slice(mc * 512, (mc + 1) * 512)
                nc.sync.dma_start(xt_sb[:, :, sl], xT_t[:, :, sl])
            wp_sb = consts.tile([128, NP, C], f16)
            nc.sync.dma_start(wp_sb[:], wp_t)
            if has_bias:
                bqk_sb = consts.tile([128, 2 * NP], f32)
                nc.sync.dma_start(bqk_sb[:], bqk_d.ap())
                bv_sb = consts.tile([128, HPC * Dh], f32)
                nc.sync.dma_start(bv_sb[:], bv_d.ap())

            Qst = consts.tile([128, NP, T], f16)        # rows 0:64 even head
            Kev = consts.tile([128, NP, T], f16)        # rows 64:128 zero
            Kod = consts.tile([128, NP, T], f16)        # rows 0:64 zero
            Vt = consts.tile([128, TKT, HPC, Dh + 1], f16)
            YT = consts.tile([128, NP, T], f16)
            # zero-fill on the scalar engine: it is idle during the initial
            # QKV phase while the DVE drains the projection copies
            nc.scalar.memzero(Kev[64:128, :, :])
            nc.scalar.memzero(Kod[0:64, :, :])
            nc.vector.memset(Vt[:, :, :, Dh:Dh + 1], 1.0)

            # ---------- QKV projection for one 512-row chunk ----------
            def qkv_groups(mc):
                msl = slice(mc * 512, (mc + 1) * 512)

                def k_group(pair):
                    ncol = 512 + pair * 128
                    ps = psum.tile([128, 512], f32, tag="ps")
                    for kt in range(KT_C):
                        nc.tensor.matmul(
                            ps[:], wqk_sb[:, kt, ncol:ncol + 128],
                            xt_sb[:, kt, msl],
                            start=(kt == 0), stop=(kt == KT_C - 1))
                    if has_bias:
                        nc.vector.tensor_scalar_add(
                            Kev[0:64, pair, msl], ps[0:64],
                            bqk_sb[0:64, NP + pair:NP + pair + 1])
                        nc.vector.tensor_scalar_add(
                            Kod[64:128, pair, msl], ps[64:128],
                            bqk_sb[64:128, NP + pair:NP + pair + 1])
                    else:
                        nc.vector.tensor_copy(Kev[0:64, pair, msl], ps[0:64])
                        nc.vector.tensor_copy(Kod[64:128, pair, msl],
                                              ps[64:128])

                def q_group(pair):
                    ncol = pair * 128
                    ps = psum.tile([128, 512], f32, tag="ps")
                    for kt in range(KT_C):
                        nc.tensor.matmul(
                            ps[:], wqk_sb[:, kt, ncol:ncol + 128],
                            xt_sb[:, kt, msl],
                            start=(kt == 0), stop=(kt == KT_C - 1))
                    if has_bias:
                        nc.vector.tensor_scalar_add(
                            Qst[:, pair, msl], ps[:],
                            bqk_sb[:, pair:pair + 1])
                    else:
                        nc.vector.tensor_copy(Qst[:, pair, msl], ps[:])

                def v_group(sub):
                    m0 = mc * 512 + sub * 128
                    ktile = 4 * mc + sub
                    vps = psum.tile([128, 512], f32, tag="ps")
                    for kt in range(KT_C):
                        nc.tensor.matmul(
                            vps[:], xt_sb[:, kt, m0:m0 + 128],
                            wv_sb[:, kt, :],
                            start=(kt == 0), stop=(kt == KT_C - 1))
                    vdst = Vt[:, ktile, :, 0:Dh]        # [128, 8, 64] strided
                    vsrc = vps[:].rearrange("p (h d) -> p h d", h=HPC)
                    if has_bias:
                        nc.vector.tensor_add(
                            vdst, vsrc,
                            bv_sb[:].rearrange("p (h d) -> p h d", h=HPC))
                    else:
                        nc.vector.tensor_copy(vdst, vsrc)

                for pair in range(NP):
                    yield lambda p=pair: k_group(p)
                for sub in range(4):
                    yield lambda s=sub: v_group(s)
                for pair in range(NP):
                    yield lambda p=pair: q_group(p)

            # ---------- projection of one 512-row chunk (after attn qc) ----
            def proj_groups(qc):
                def pgroup(mt, nh):
                    pp2 = psum.tile([128, 512], f32, tag="ps")
                    for dg in range(NP):
                        nc.tensor.matmul(
                            pp2[:], YT[:, dg, mt * 128:(mt + 1) * 128],
                            wp_sb[:, dg, nh * 512:(nh + 1) * 512],
                            start=(dg == 0), stop=(dg == NP - 1))
                    ot = obuf.tile([128, 512], f16, tag="ot", bufs=4)
                    if (mt + nh) % 2:
                        nc.scalar.copy(ot[:], pp2[:])
                    else:
                        nc.vector.tensor_copy(ot[:], pp2[:])
                    nc.sync.dma_start(
                        out_d.ap()[mt * 128:(mt + 1) * 128,
                                   nh * 512:(nh + 1) * 512], ot[:])

                for mt in range(4 * qc, 4 * qc + 4):
                    for nh in range(2):
                        yield lambda m=mt, n=nh: pgroup(m, n)

            # ---------- attention for one query chunk, with bg interleave --
            def attention(qc, background):
                q_sl = slice(qc * 512, (qc + 1) * 512)
                nkt = 4 * (qc + 1)
                for hp in range(NP):
                    yt0 = psyt.tile([65, 512], f32, tag="yt")
                    yt1 = psyt.tile([65, 512], f32, tag="yt")
                    for kt in range(nkt):
                        k_sl = slice(kt * 128, (kt + 1) * 128)
                        r = kt - 4 * qc
                        # queries before 128*r in this chunk see no valid key
                        # in a diagonal tile: skip those moving columns.
                        q0 = max(r, 0) * 128 if DIAG_RESTRICT else 0
                        qr = slice(qc * 512 + q0, (qc + 1) * 512)
                        cr = slice(q0, 512)
                        stp = psst.tile([128, 2, 512], f32, tag="st")
                        nc.tensor.matmul(stp[:, 0, cr], Kev[:, hp, k_sl],
                                         Qst[:, hp, qr],
                                         start=True, stop=True)
                        nc.tensor.matmul(stp[:, 1, cr], Kod[:, hp, k_sl],
                                         Qst[:, hp, qr],
                                         start=True, stop=True)
                        pp = pbuf.tile([128, 2, 512], f16, tag="pp", bufs=8)
                        nc.scalar.activation(pp[:, :, cr], stp[:, :, cr],
                                             EXP, scale=SCALE)
                        if r >= 0:
                            # only the 128-wide diagonal block needs masking
                            # (keys in this tile are all valid for later
                            # queries); the pattern is the same lower
                            # triangle for every r
                            br = slice(128 * r, 128 * r + 128)
                            nc.vector.tensor_mul(pp[:, :, br], pp[:, :, br],
                                                 mask_sb[:])
                        first, last = (kt == 0), (kt == nkt - 1)
                        nc.tensor.matmul(yt0[:, cr], Vt[:, kt, 2 * hp, :],
                                         pp[:, 0, cr], start=first, stop=last,
                                         skip_group_check=True)
                        nc.tensor.matmul(yt1[:, cr], Vt[:, kt, 2 * hp + 1, :],
                                         pp[:, 1, cr], start=first, stop=last,
                                         skip_group_check=True)
                        if background:
                            background.pop(0)()
                    # Free the yt accumulators with copies, then normalize
                    # YT in place.  partition_broadcast only writes correctly
                    # at base partition 0, so broadcast both heads there and
                    # shift-copy the odd half up with the DVE.  The very last
                    # boundary instead multiplies straight out of PSUM (the
                    # projection tail waits on it).
                    last = (qc == QC - 1 and hp == NP - 1)
                    if not last:
                        bc = work.tile([128, 512], f32, tag="bc", bufs=2)
                        bcx = work.tile([64, 512], f32, tag="bcx", bufs=2)
                        for h, ytp in ((0, yt0), (1, yt1)):
                            y_sl = (slice(h * 64, (h + 1) * 64), hp, q_sl)
                            zr = work.tile([1, 512], f32, tag="zr", bufs=4)
                            nc.vector.tensor_copy(zr[:], ytp[64:65, :])
                            nc.vector.tensor_copy(YT[y_sl], ytp[0:64, :])
                            zi = work.tile([1, 512], f32, tag="zi", bufs=4)
                            nc.vector.reciprocal_approx_fast(zi[:], zr[:])
                            nc.gpsimd.partition_broadcast(
                                bc[0:64, :] if h == 0 else bcx[:], zi[:])
                        nc.vector.tensor_copy(bc[64:128, :], bcx[:])
                        nc.vector.tensor_mul(YT[:, hp, q_sl],
                                             YT[:, hp, q_sl], bc[:])
                    else:
                        # final boundary gates the projection tail: multiply
                        # straight out of PSUM in column halves so proj(mt)
                        # for the first half can start earlier
                        for ci in range(2):
                            cs = slice(ci * 256, (ci + 1) * 256)
                            qs = slice(qc * 512 + ci * 256,
                                       qc * 512 + (ci + 1) * 256)
                            for h, ytp in ((0, yt0), (1, yt1)):
                                y_sl = (slice(h * 64, (h + 1) * 64), hp, qs)
                                zr = work.tile([1, 256], f32, tag="zr",
                                               bufs=4)
                                nc.vector.tensor_copy(zr[:], ytp[64:65, cs])
                                zi = work.tile([1, 256], f32, tag="zi",
                                               bufs=4)
                                nc.vector.reciprocal_approx_fast(zi[:], zr[:])
                                bch = work.tile([64, 256], f32, tag="bch",
                                                bufs=4)
                                nc.gpsimd.partition_broadcast(bch[:], zi[:])
                                nc.vector.tensor_mul(YT[y_sl], ytp[0:64, cs],
                                                     bch[:])

            # ---------------- main schedule ----------------
            # attn(qc) is paced by ACT exp; fill the PE with qkv chunk qc+1,
            # and all three early projections during the long attn(3).
            for g in qkv_groups(0):
                g()
            for qc in range(QC):
                background = []
                if qc + 1 < QC:
                    background.extend(qkv_groups(qc + 1))
                else:
                    for p in range(QC - 1):
                        background.extend(proj_groups(p))
                attention(qc, background)
                for g in background:                     # leftovers
                    g()
            for g in proj_groups(QC - 1):
                g()
            if DEBUG:
                nc.sync.dma_start(dbg_yt_d.ap(), YT[:])
                nc.sync.dma_start(dbg_q_d.ap(), Qst[:])
                nc.sync.dma_start(dbg_k_d.ap(), Kev[:])
                nc.sync.dma_start(dbg_v_d.ap(), Vt[:])

    nc.compile()
    return nc


def _get_nc(has_bias: bool):
    key = ("nc", has_bias)
    if key not in _cache:
        _cache[key] = _build(has_bias)
    return _cache[key]


def _make_masks() -> np.ndarray:
    # masks[p, h, j] = 1.0 where key-offset p <= query-offset j inside the
    # 128x128 diagonal block (identical pattern for every diagonal tile)
    p = np.arange(128)[:, None]
    j = np.arange(128)[None, :]
    m = (p <= j).astype(np.float16)                       # [128, 128]
    return np.ascontiguousarray(np.repeat(m[:, None, :], 2, axis=1))


def kernel(x, W_qkv, b_qkv, W_proj, b_proj):
    from concourse.bass_utils import run_bass_kernel_spmd

    x = np.asarray(x, dtype=np.float32)
    W_qkv = np.asarray(W_qkv, dtype=np.float32)
    b_qkv = np.asarray(b_qkv, dtype=np.float32)
    W_proj = np.asarray(W_proj, dtype=np.float32)
    b_proj = np.asarray(b_proj, dtype=np.float32)

    has_bias = bool(np.any(b_qkv != 0.0))
    nc = _get_nc(has_bias)

    masks = _make_masks()
    GW = HPC * Dh                                         # 512 channels/group
    in_maps = []
    for c in range(NCORES):
        b, g = divmod(c, 2)
        h0 = g * GW
        xT = np.ascontiguousarray(x[b].T.astype(np.float16))   # [C, T]
        w_q = W_qkv[:, h0:h0 + GW]
        w_k = W_qkv[:, C + h0:C + h0 + GW]
        w_v = W_qkv[:, 2 * C + h0:2 * C + h0 + GW]
        b_q = b_qkv[h0:h0 + GW]
        b_k = b_qkv[C + h0:C + h0 + GW]
        b_v = b_qkv[2 * C + h0:2 * C + h0 + GW]
        in_maps.append({
            "xT": xT,
            "w_qk": np.ascontiguousarray(
                np.concatenate([w_q, w_k], axis=1).astype(np.float16)),
            "w_v": np.ascontiguousarray(w_v.astype(np.float16)),
            "w_p": np.ascontiguousarray(
                W_proj[h0:h0 + GW, :].astype(np.float16)),
            "masks": masks,
            "b_qk": np.ascontiguousarray(
                np.concatenate([b_q.reshape(NP, 128).T,
                                b_k.reshape(NP, 128).T], axis=1)),
            "b_v_row": np.ascontiguousarray(
                np.broadcast_to(b_v[None, :], (128, GW)).astype(np.float32)),
        })

    res = run_bass_kernel_spmd(nc, in_maps, core_ids=list(range(NCORES)),
                               **_cache.get("run_kwargs", {}))
    _cache["last_results"] = res

    out = np.empty((B, T, C), dtype=np.float32)
    for b in range(B):
        out[b] = (res.results[2 * b]["out_p"].astype(np.float32)
                  + res.results[2 * b + 1]["out_p"].astype(np.float32)
                  + b_proj[None, :])
    return out

